# revision 1
# baseline (speedup 1.0000x reference)
"""Bass/Trainium2 kernel for nn_F_Loss_65446711656630.

Strategy (data-parallel over N, 8 cores):
  - Host: GLOBAL stable sort of all rows by class id, then slice 8192 rows
    per core and transpose to [512 features x 8192 rows] contiguous pieces.
    After a global sort each core spans only ~2 classes, so class
    transitions are rare at any granularity.
  - Device (static kernel): stream 16 fp16 pieces of [128, 2048]; per piece
      * DVE:  per-128-row-chunk partial sums of h (one multi-chunk
              TensorReduce per piece, 3D access pattern)
      * ACT:  square with accum_out -> per-piece partial sum of h^2
              (the square pass itself yields the sumsq reduction, so no
              second DVE reduce pass is needed)
    Inputs ship as fp16 (costs ~1e-5 final rel err, halves HBM traffic to
    8 MiB/core); the per-element engine passes (~36-39 us) are the cap,
    with DMA at ~23 us well underneath.
  - Host: per-class stats from single-class chunk/piece partials (fp64)
    + direct numpy sums for the few transition chunks/pieces; then the
    tiny O(C^2 D) pairwise betainc/top-k stage in f32 jax on CPU
    (mirroring the reference's numerics exactly).
"""

import numpy as np

C = 16
D = 512
N = 65536
NCORES = 8
ROWS = N // NCORES          # 8192 rows per core
P = 128                     # SBUF partitions
PIECE = 2048                # rows per DMA piece / sumsq granule
X = 128                     # rows per sums granule (DVE reduce chunk)
NBLK = D // P               # 4 feature blocks
NPIECE = ROWS // PIECE      # 4 pieces per block
NCHUNK = ROWS // X          # 64 chunks per core
CPP = PIECE // X            # 8 chunks per piece
XMIN, XMAX = 1e-37, 1.0 - 1e-5

_NC_CACHE = {}


def _build_nc():
    """Per-core SPMD program.

    Inputs:  "ht"   [16, 128, 2048] fp16 (piece (b,p) at index b*4+p holds
                                         features b*128..+128 x rows
                                         p*2048..+2048, contiguous)
    Outputs: "hsum" [128, 256] f32  (hsum[f, b*64+g] = sum over 128-row
                                     chunk g of feature b*128+f)
             "ssum" [128, 16]  f32  (ssum[f, b*4+p]  = sum over piece p's
                                     2048 rows of feature (b*128+f)^2)
    """
    import concourse.tile as tile
    from concourse import bacc, mybir

    f32 = mybir.dt.float32

    nc = bacc.Bacc("TRN2", target_bir_lowering=False, debug=False,
                   num_devices=NCORES)
    f16 = mybir.dt.float16
    ht = nc.declare_dram_parameter("ht", [NBLK * NPIECE, P, PIECE], f16,
                                   isOutput=False)
    hsum = nc.declare_dram_parameter("hsum", [P, NBLK * NCHUNK], f32, isOutput=True)
    ssum = nc.declare_dram_parameter("ssum", [P, NBLK * NPIECE], f32, isOutput=True)

    with tile.TileContext(nc) as tc:
        with (
            tc.tile_pool(name="pc", bufs=8) as piece_pool,
            tc.tile_pool(name="sq", bufs=3) as sq_pool,
            tc.tile_pool(name="acc", bufs=1) as acc_pool,
        ):
            hpart = acc_pool.tile([P, NBLK * NCHUNK], f32, tag="hpart")
            spart = acc_pool.tile([P, NBLK * NPIECE], f32, tag="spart")

            for i in range(NBLK * NPIECE):
                t = piece_pool.tile([P, PIECE], f16)
                nc.sync.dma_start(t[:], ht[i])

                # ACT: square (scratch) + free-dim accumulate -> piece sumsq
                sq = sq_pool.tile([P, PIECE], f32)
                nc.scalar.activation(
                    sq[:], t[:], mybir.ActivationFunctionType.Square,
                    accum_out=spart[:, i:i + 1])

                # DVE: one multi-chunk reduce -> chunk sums of h
                base = i * CPP
                t3 = t[:].rearrange("p (c x) -> p c x", x=X)
                nc.vector.reduce_sum(
                    hpart[:, base:base + CPP], t3, axis=mybir.AxisListType.X)

            nc.sync.dma_start(hsum[:], hpart[:])
            nc.sync.dma_start(ssum[:], spart[:])
    nc.compile()
    return nc


def _get_nc():
    if "nc" not in _NC_CACHE:
        _NC_CACHE["nc"] = _build_nc()
    return _NC_CACHE["nc"]


def _granule_classes(ids_sorted, size):
    """Per-granule class id, or -1 if the granule spans a class boundary."""
    g = ids_sorted.reshape(-1, size)
    pure = g[:, 0] == g[:, -1]
    return np.where(pure, g[:, 0], -1).astype(np.int64)


def _prep_core(hs_k, ids_k):
    """hs_k/ids_k already globally sorted. Returns device input + host fixups."""
    T = np.ascontiguousarray(
        hs_k.reshape(NPIECE, PIECE, NBLK, P).transpose(2, 0, 3, 1)
        .astype(np.float16)
    ).reshape(NBLK * NPIECE, P, PIECE)           # [16, 128, 2048] fp16

    chunk_cls = _granule_classes(ids_k, X)       # [64]
    piece_cls = _granule_classes(ids_k, PIECE)   # [8]

    bsum = np.zeros((C, D), dtype=np.float64)
    bsq = np.zeros((C, D), dtype=np.float64)
    # transition chunks: host computes their per-class h sums
    if (chunk_cls < 0).any():
        m = np.repeat(chunk_cls < 0, X)
        rows, rids = hs_k[m].astype(np.float64), ids_k[m]
        for q in np.unique(rids):
            bsum[q] += rows[rids == q].sum(axis=0)
    # transition pieces: host computes their per-class h^2 sums
    if (piece_cls < 0).any():
        m = np.repeat(piece_cls < 0, PIECE)
        rows, rids = hs_k[m].astype(np.float64), ids_k[m]
        for q in np.unique(rids):
            sel = rows[rids == q]
            bsq[q] += (sel * sel).sum(axis=0)
    return T, chunk_cls, piece_cls, bsum, bsq


def _device_stats(hidden, ids, **run_kwargs):
    """Returns (sums[C,D], sumsq[C,D]) float64, plus the raw run result."""
    from concourse import bass_utils

    nc = _get_nc()

    order = np.argsort(ids, kind="stable")       # GLOBAL sort by class
    ids_s = ids[order]
    hs = hidden[order]

    in_maps = []
    meta = []
    sums = np.zeros((C, D), dtype=np.float64)
    sumsq = np.zeros((C, D), dtype=np.float64)
    for k in range(NCORES):
        rows = slice(k * ROWS, (k + 1) * ROWS)
        T, ccls, pcls, bsum, bsq = _prep_core(hs[rows], ids_s[rows])
        in_maps.append({"ht": T})
        meta.append((ccls, pcls))
        sums += bsum
        sumsq += bsq

    res = bass_utils.run_bass_kernel_spmd(nc, in_maps, list(range(NCORES)), **run_kwargs)

    eye = np.arange(C)[None, :]
    for k in range(NCORES):
        ccls, pcls = meta[k]
        hp = res.results[k]["hsum"].astype(np.float64)
        sp = res.results[k]["ssum"].astype(np.float64)
        # [128, b, g] -> [g, b, 128] -> [granule, feature]
        hp = hp.reshape(P, NBLK, NCHUNK).transpose(2, 1, 0).reshape(NCHUNK, D)
        sp = sp.reshape(P, NBLK, NPIECE).transpose(2, 1, 0).reshape(NPIECE, D)
        cm = ccls >= 0
        sums += ((ccls[cm, None] == eye).astype(np.float64)).T @ hp[cm]
        pm = pcls >= 0
        sumsq += ((pcls[pm, None] == eye).astype(np.float64)).T @ sp[pm]
    return sums, sumsq, res


def _pairwise_loss(counts, sums, sumsq, d):
    """The tiny O(C^2 D) stage on host CPU.

    Runs in float32 with the same jax ops as the reference: at these extreme
    betainc parameters (b ~ 8190, x ~ 1e-5) jax's f32 betainc differs from
    the true (f64) value by ~1e-3, so matching the reference requires
    replicating its f32 numerics, not improving on them.
    """
    import jax
    import jax.numpy as jnp

    cpu = jax.devices("cpu")[0]
    with jax.default_device(cpu):
        counts64 = counts.astype(np.float64)
        means64 = sums / counts64[:, None]
        withins64 = sumsq - counts64[:, None] * means64**2
        counts = jnp.asarray(counts64, jnp.float32)               # [C]
        means = jnp.asarray(means64, jnp.float32)                 # [C, D]
        withins = jnp.asarray(withins64, jnp.float32)             # [C, D]
        half_diff = (means[:, None, :] - means[None, :, :]) * 0.5
        pair_counts = counts[:, None] + counts[None, :]
        pair_between = half_diff * half_diff * pair_counts[:, :, None]
        pair_within = withins[:, None, :] + withins[None, :, :]
        d2 = pair_counts - 2.0
        d2 = jnp.where(d2 == 0.0, 1e-5, d2)
        x = pair_between / (pair_between + pair_within)
        x = jnp.clip(x, XMIN, XMAX)
        a = jnp.full_like(x, 0.5)
        b = jnp.broadcast_to((d2 * 0.5)[:, :, None], x.shape)
        xbetainc = jax.scipy.special.betainc(a, b, x)             # [C, C, D]
        top_k, _ = jax.lax.top_k(xbetainc, int(d))                # [C, C, d]
        per_pair = jnp.sum(jnp.log(top_k), axis=-1)               # [C, C]
        mask = jnp.triu(jnp.ones((C, C), dtype=bool), k=1)
        total = jnp.sum(jnp.where(mask, per_pair, jnp.zeros_like(per_pair)))
        return float(-total)


def kernel(hidden, batch_ids, d):
    hidden = np.asarray(hidden, dtype=np.float32)
    ids = np.asarray(batch_ids).astype(np.int64)
    assert hidden.shape == (N, D), hidden.shape

    counts = np.bincount(ids, minlength=C).astype(np.float64)
    sums, sumsq, _ = _device_stats(hidden, ids)
    total = _pairwise_loss(counts, sums, sumsq, int(np.asarray(d)))
    return np.array(total, dtype=np.float32)



# revision 10
# speedup vs baseline: 1.4569x; 1.4569x over previous
"""Bass/Trainium2 kernel for nn_F_Loss_65446711656630.

Strategy (data-parallel over N, 8 cores):
  - Host: GLOBAL stable sort of all rows by class id, quantize to fp8 e4m3
    (final loss rel err ~2e-4, well under tolerance; halves HBM traffic vs
    fp16 to ~4.2 MiB/core), then lay out per-core DoubleRow matmul operands:
    [granule, partition, chunkpair, two, 4*(128 feat + ones col)].
  - Device (static kernel): ALL arithmetic on the TensorEngine. For each
    128-row chunk and each 128-feature block b, one fp8 matmul with
    stationary = X_b and moving = [X_b | 1] accumulates X_b^T X_b (diag =
    per-feature sum of squares) and X_b^T 1 (per-feature sums) into PSUM.
    (DoubleRow is a loss here: it disables the compiler's Fast Weight Load
    and pays a 256-column LDWEIGHTS per 129-column matmul.) 16 chunks
    accumulate per 2048-row granule; granule stats stage through SBUF to
    HBM. DVE is not used at all; PE ~14us overlapping ~13us of DMA.
  - Host: per-class stats from single-class granule partials (f64) + direct
    numpy f64 sums for the few class-transition granules; then the tiny
    O(C^2 D) pairwise betainc/top-k stage in f32 jax on CPU (mirroring the
    reference's numerics exactly).
"""

import ml_dtypes
import numpy as np

C = 16
D = 512
N = 65536
NCORES = 8
ROWS = N // NCORES          # 8192 rows per core
P = 128                     # SBUF partitions
NBLK = D // P               # 4 feature blocks
BCOL = P + 1                # 129 columns per block: 128 features + ones col
GRAN = 2048                 # rows per granule (PSUM accumulation unit)
NGRAN = ROWS // GRAN        # 4 granules per core
NCHK = GRAN // P            # 16 chunks (matmuls) per granule per block
XMIN, XMAX = 1e-37, 1.0 - 1e-5

F8 = ml_dtypes.float8_e4m3

_NC_CACHE = {}


def _build_nc():
    """Per-core SPMD program.

    Inputs:  "ht"    [4, 128, 16, 516] fp8e4
             (granule, partition, chunk, 4 blocks x [128 features | 1.0];
              row r within granule = chunk*128 + p)
    Outputs: "stats" [4, 128, 4, 129] f32
             (stats[g, f, b, c] = sum over granule g's rows of
              X[:, b*128+f] * X[:, b*128+c] for c < 128, and the plain sum
              of X[:, b*128+f] for c == 128; only the Gram diagonal c == f
              and the sums column are consumed by the host)
    """
    import concourse.tile as tile
    from concourse import bacc, mybir

    f32 = mybir.dt.float32
    f8 = mybir.dt.float8e4

    nc = bacc.Bacc("TRN2", target_bir_lowering=False, debug=False,
                   num_devices=NCORES)
    ht = nc.declare_dram_parameter("ht", [NGRAN, P, NCHK, NBLK * BCOL], f8,
                                   isOutput=False)
    stats = nc.declare_dram_parameter("stats", [NGRAN, P, NBLK, BCOL], f32,
                                      isOutput=True)

    with tile.TileContext(nc) as tc:
        with (
            tc.tile_pool(name="in", bufs=NGRAN) as in_pool,
            tc.tile_pool(name="st", bufs=2) as stage_pool,
            tc.tile_pool(name="ps", bufs=2, space="PSUM") as psum_pool,
        ):
            tiles = []
            for g in range(NGRAN):
                t = in_pool.tile([P, NCHK, NBLK * BCOL], f8, tag=f"t{g}")
                nc.sync.dma_start(t[:], ht[g])
                tiles.append(t)

            for g in range(NGRAN):
                t3 = tiles[g][:]
                # one 4-bank psum tile per granule; block b owns bank b
                pt = psum_pool.tile([P, NBLK, 512], f32, tag="ps")
                for b in range(NBLK):
                    for ch in range(NCHK):
                        stat_ap = t3[:, ch, b * BCOL:b * BCOL + P]
                        mov_ap = t3[:, ch, b * BCOL:b * BCOL + BCOL]
                        nc.tensor.matmul(
                            pt[:, b, 0:BCOL], stat_ap, mov_ap,
                            start=(ch == 0), stop=(ch == NCHK - 1))
                stage = stage_pool.tile([P, NBLK, BCOL], f32, tag="st")
                nc.scalar.copy(stage[:], pt[:, :, 0:BCOL])
                nc.sync.dma_start(stats[g], stage[:])
    nc.compile()
    return nc


def _get_nc():
    if "nc" not in _NC_CACHE:
        _NC_CACHE["nc"] = _build_nc()
    return _NC_CACHE["nc"]


def _granule_classes(ids_sorted, size):
    """Per-granule class id, or -1 if the granule spans a class boundary."""
    g = ids_sorted.reshape(-1, size)
    pure = g[:, 0] == g[:, -1]
    return np.where(pure, g[:, 0], -1).astype(np.int64)


def _prep_core(hs_k, ids_k):
    """hs_k/ids_k already globally sorted. Returns device input + host fixups."""
    q5 = hs_k.astype(F8).reshape(NGRAN, NCHK, P, NBLK, P)
    buf = np.empty((NGRAN, P, NCHK, NBLK, BCOL), dtype=F8)
    buf[..., :P] = q5.transpose(0, 2, 1, 3, 4)
    buf[..., P] = np.array(1.0, dtype=F8)
    ht = buf.reshape(NGRAN, P, NCHK, NBLK * BCOL)

    gcls = _granule_classes(ids_k, GRAN)          # [4]

    bsum = np.zeros((C, D), dtype=np.float64)
    bsq = np.zeros((C, D), dtype=np.float64)
    # transition granules: host computes their per-class stats exactly
    if (gcls < 0).any():
        m = np.repeat(gcls < 0, GRAN)
        rows, rids = hs_k[m].astype(np.float64), ids_k[m]
        for q in np.unique(rids):
            sel = rows[rids == q]
            bsum[q] += sel.sum(axis=0)
            bsq[q] += (sel * sel).sum(axis=0)
    return ht, gcls, bsum, bsq


def _device_stats(hidden, ids, **run_kwargs):
    """Returns (sums[C,D], sumsq[C,D]) float64, plus the raw run result."""
    from concourse import bass_utils

    nc = _get_nc()

    order = np.argsort(ids, kind="stable")       # GLOBAL sort by class
    ids_s = ids[order]
    hs = hidden[order]

    in_maps = []
    meta = []
    sums = np.zeros((C, D), dtype=np.float64)
    sumsq = np.zeros((C, D), dtype=np.float64)
    for k in range(NCORES):
        rows = slice(k * ROWS, (k + 1) * ROWS)
        ht, gcls, bsum, bsq = _prep_core(hs[rows], ids_s[rows])
        in_maps.append({"ht": ht})
        meta.append(gcls)
        sums += bsum
        sumsq += bsq

    res = bass_utils.run_bass_kernel_spmd(nc, in_maps, list(range(NCORES)),
                                          **run_kwargs)

    for k in range(NCORES):
        gcls = meta[k]
        st = res.results[k]["stats"].astype(np.float64)  # [4, 128, 4, 129]
        # [g, f, b] -> [g, b, f] -> [g, 512] (feature id = b*128 + f)
        gsums = st[:, :, :, P].transpose(0, 2, 1).reshape(NGRAN, D)
        gdiag = np.diagonal(st[:, :, :, :P], axis1=1, axis2=3)  # [g, b, f]
        gsq = gdiag.reshape(NGRAN, D)
        for g in range(NGRAN):
            c = gcls[g]
            if c >= 0:
                sums[c] += gsums[g]
                sumsq[c] += gsq[g]
    return sums, sumsq, res


def _pairwise_loss(counts, sums, sumsq, d):
    """The tiny O(C^2 D) stage on host CPU.

    Runs in float32 with the same jax ops as the reference: at these extreme
    betainc parameters (b ~ 8190, x ~ 1e-5) jax's f32 betainc differs from
    the true (f64) value by ~1e-3, so matching the reference requires
    replicating its f32 numerics, not improving on them.
    """
    import jax
    import jax.numpy as jnp

    cpu = jax.devices("cpu")[0]
    with jax.default_device(cpu):
        counts64 = counts.astype(np.float64)
        means64 = sums / counts64[:, None]
        withins64 = sumsq - counts64[:, None] * means64**2
        counts = jnp.asarray(counts64, jnp.float32)               # [C]
        means = jnp.asarray(means64, jnp.float32)                 # [C, D]
        withins = jnp.asarray(withins64, jnp.float32)             # [C, D]
        half_diff = (means[:, None, :] - means[None, :, :]) * 0.5
        pair_counts = counts[:, None] + counts[None, :]
        pair_between = half_diff * half_diff * pair_counts[:, :, None]
        pair_within = withins[:, None, :] + withins[None, :, :]
        d2 = pair_counts - 2.0
        d2 = jnp.where(d2 == 0.0, 1e-5, d2)
        x = pair_between / (pair_between + pair_within)
        x = jnp.clip(x, XMIN, XMAX)
        a = jnp.full_like(x, 0.5)
        b = jnp.broadcast_to((d2 * 0.5)[:, :, None], x.shape)
        xbetainc = jax.scipy.special.betainc(a, b, x)             # [C, C, D]
        top_k, _ = jax.lax.top_k(xbetainc, int(d))                # [C, C, d]
        per_pair = jnp.sum(jnp.log(top_k), axis=-1)               # [C, C]
        mask = jnp.triu(jnp.ones((C, C), dtype=bool), k=1)
        total = jnp.sum(jnp.where(mask, per_pair, jnp.zeros_like(per_pair)))
        return float(-total)


def kernel(hidden, batch_ids, d):
    hidden = np.asarray(hidden, dtype=np.float32)
    ids = np.asarray(batch_ids).astype(np.int64)
    assert hidden.shape == (N, D), hidden.shape

    counts = np.bincount(ids, minlength=C).astype(np.float64)
    sums, sumsq, _ = _device_stats(hidden, ids)
    total = _pairwise_loss(counts, sums, sumsq, int(np.asarray(d)))
    return np.array(total, dtype=np.float32)


# revision 12
# speedup vs baseline: 1.4854x; 1.0196x over previous
"""Bass/Trainium2 kernel for nn_F_Loss_65446711656630.

Strategy (data-parallel over N, 8 cores):
  - Host: GLOBAL stable sort of all rows by class id, quantize to fp8 e4m3
    (final loss rel err ~2e-4, well under tolerance; halves HBM traffic vs
    fp16 to ~4.2 MiB/core), then lay out per-core matmul operands:
    [granule, partition, chunk, 4*(128 feat + ones col)].
  - Device (static kernel), work split across engines:
      * TensorE (blocks 0-2): for each 128-row chunk, one fp8 matmul with
        stationary = X_b and moving = [X_b | 1] accumulates X_b^T X_b
        (diag = per-feature sum of squares) and X_b^T 1 (per-feature sums)
        into PSUM. (DoubleRow loses here: it disables Fast Weight Load and
        pays a 256-column LDWEIGHTS per 129-column matmul.) 16 chunks
        accumulate per 2048-row granule; granule stats stage through SBUF.
      * ACT (block 3): Square activation with accum_out -> half-granule
        sum of squares.
      * DVE (block 3): TensorReduce -> per-chunk sums.
    PE ~13us, ACT ~10us, DVE ~10us, all overlapping ~13us of DMA.
  - Host: per-class stats from single-class granule partials (f64) + direct
    numpy f64 sums for the few class-transition granules; then the tiny
    O(C^2 D) pairwise betainc/top-k stage in f32 jax on CPU (mirroring the
    reference's numerics exactly).
"""

import ml_dtypes
import numpy as np

C = 16
D = 512
N = 65536
NCORES = 8
ROWS = N // NCORES          # 8192 rows per core
P = 128                     # SBUF partitions
NBLK = 4                    # feature blocks of 128
PEBLK = 3                   # blocks computed on the TensorEngine
BCOL = P + 1                # 129 columns per block: 128 features + ones col
GRAN = 2048                 # rows per granule (stats accumulation unit)
NGRAN = ROWS // GRAN        # 4 granules per core
NCHK = GRAN // P            # 16 chunks per granule
HCHK = NCHK // 2            # 8 chunks per half-granule DMA
NHALF = NGRAN * 2
XMIN, XMAX = 1e-37, 1.0 - 1e-5

F8 = ml_dtypes.float8_e4m3

_NC_CACHE = {}


def _build_nc():
    """Per-core SPMD program.

    Inputs:  "ht"    [4, 128, 2, 8, 516] fp8e4
             (granule, partition, half, chunk, 4 blocks x [128 feat | 1.0];
              row r within granule = (half*8 + chunk)*128 + p)
    Outputs: "stats" [4, 128, 3, 129] f32   (PE blocks 0-2: stats[g,f,b,c] =
               sum over granule g of X[:,b*128+f]*X[:,b*128+c] for c<128,
               plain sum of X[:,b*128+f] at c==128)
             "sq3"   [128, 8] f32   (ACT: half-granule sumsq of feature
               384+p; granule g = cols 2g and 2g+1)
             "sm3"   [128, 64] f32  (DVE: chunk sums of feature 384+p;
               granule g = cols 16g..16g+15)
    """
    import concourse.tile as tile
    from concourse import bacc, mybir

    f32 = mybir.dt.float32
    f8 = mybir.dt.float8e4

    nc = bacc.Bacc("TRN2", target_bir_lowering=False, debug=False,
                   num_devices=NCORES)
    ht = nc.declare_dram_parameter("ht", [NGRAN, P, 2, HCHK, NBLK * BCOL], f8,
                                   isOutput=False)
    stats = nc.declare_dram_parameter("stats", [NGRAN, P, PEBLK, BCOL], f32,
                                      isOutput=True)
    sq3 = nc.declare_dram_parameter("sq3", [P, NHALF], f32, isOutput=True)
    sm3 = nc.declare_dram_parameter("sm3", [P, NGRAN * NCHK], f32,
                                    isOutput=True)
    B3 = PEBLK * BCOL  # column offset of block 3

    with tile.TileContext(nc) as tc:
        with (
            tc.tile_pool(name="in", bufs=1) as in_pool,
            tc.tile_pool(name="st", bufs=2) as stage_pool,
            tc.tile_pool(name="sc", bufs=2) as scr_pool,
            tc.tile_pool(name="acc", bufs=1) as acc_pool,
            tc.tile_pool(name="ps", bufs=2, space="PSUM") as psum_pool,
        ):
            sq3_t = acc_pool.tile([P, NHALF], f32, tag="sq3")
            sm3_t = acc_pool.tile([P, NGRAN * NCHK], f32, tag="sm3")

            tiles = []
            for g in range(NGRAN):
                for h in range(2):
                    t = in_pool.tile([P, HCHK, NBLK * BCOL], f8,
                                     tag=f"t{g}{h}")
                    nc.sync.dma_start(t[:], ht[g][:, h])
                    tiles.append(t)

            for g in range(NGRAN):
                # block 3 on ACT (sumsq) + DVE (sums), per half-granule
                for h in range(2):
                    th = tiles[2 * g + h][:]
                    b3 = th[:, :, B3:B3 + P]             # [128, 8, 128] fp8
                    scr = scr_pool.tile([P, HCHK, P], f32, tag="scr")
                    nc.scalar.activation(
                        scr[:], b3, mybir.ActivationFunctionType.Square,
                        accum_out=sq3_t[:, 2 * g + h:2 * g + h + 1])
                    nc.vector.reduce_sum(
                        sm3_t[:, g * NCHK + h * HCHK:g * NCHK + (h + 1) * HCHK],
                        b3, axis=mybir.AxisListType.X)

                # blocks 0-2 on the TensorEngine
                pt = psum_pool.tile([P, PEBLK, 512], f32, tag="ps")
                stage = stage_pool.tile([P, PEBLK, BCOL], f32, tag="st")
                for b in range(PEBLK):
                    for ch in range(NCHK):
                        th = tiles[2 * g + ch // HCHK][:]
                        lc = ch % HCHK
                        stat_ap = th[:, lc, b * BCOL:b * BCOL + P]
                        mov_ap = th[:, lc, b * BCOL:b * BCOL + BCOL]
                        nc.tensor.matmul(
                            pt[:, b, 0:BCOL], stat_ap, mov_ap,
                            start=(ch == 0), stop=(ch == NCHK - 1))
                    nc.scalar.copy(stage[:, b], pt[:, b, 0:BCOL])
                nc.sync.dma_start(stats[g], stage[:])

            nc.sync.dma_start(sq3[:], sq3_t[:])
            nc.sync.dma_start(sm3[:], sm3_t[:])
    nc.compile()
    return nc


def _get_nc():
    if "nc" not in _NC_CACHE:
        _NC_CACHE["nc"] = _build_nc()
    return _NC_CACHE["nc"]


def _granule_classes(ids_sorted, size):
    """Per-granule class id, or -1 if the granule spans a class boundary."""
    g = ids_sorted.reshape(-1, size)
    pure = g[:, 0] == g[:, -1]
    return np.where(pure, g[:, 0], -1).astype(np.int64)


def _prep_core(hs_k, ids_k):
    """hs_k/ids_k already globally sorted. Returns device input + host fixups."""
    q5 = hs_k.astype(F8).reshape(NGRAN, NCHK, P, NBLK, P)
    buf = np.empty((NGRAN, P, NCHK, NBLK, BCOL), dtype=F8)
    buf[..., :P] = q5.transpose(0, 2, 1, 3, 4)
    buf[..., P] = np.array(1.0, dtype=F8)
    ht = buf.reshape(NGRAN, P, 2, HCHK, NBLK * BCOL)

    gcls = _granule_classes(ids_k, GRAN)          # [4]

    bsum = np.zeros((C, D), dtype=np.float64)
    bsq = np.zeros((C, D), dtype=np.float64)
    # transition granules: host computes their per-class stats exactly
    if (gcls < 0).any():
        m = np.repeat(gcls < 0, GRAN)
        rows, rids = hs_k[m].astype(np.float64), ids_k[m]
        for q in np.unique(rids):
            sel = rows[rids == q]
            bsum[q] += sel.sum(axis=0)
            bsq[q] += (sel * sel).sum(axis=0)
    return ht, gcls, bsum, bsq


def _device_stats(hidden, ids, **run_kwargs):
    """Returns (sums[C,D], sumsq[C,D]) float64, plus the raw run result."""
    from concourse import bass_utils

    nc = _get_nc()

    order = np.argsort(ids, kind="stable")       # GLOBAL sort by class
    ids_s = ids[order]
    hs = hidden[order]

    in_maps = []
    meta = []
    sums = np.zeros((C, D), dtype=np.float64)
    sumsq = np.zeros((C, D), dtype=np.float64)
    for k in range(NCORES):
        rows = slice(k * ROWS, (k + 1) * ROWS)
        ht, gcls, bsum, bsq = _prep_core(hs[rows], ids_s[rows])
        in_maps.append({"ht": ht})
        meta.append(gcls)
        sums += bsum
        sumsq += bsq

    res = bass_utils.run_bass_kernel_spmd(nc, in_maps, list(range(NCORES)),
                                          **run_kwargs)

    DPE = PEBLK * P  # 384 features on the PE path
    for k in range(NCORES):
        gcls = meta[k]
        st = res.results[k]["stats"].astype(np.float64)  # [4, 128, 3, 129]
        # [g, f, b] -> [g, b, f] -> [g, 384] (feature id = b*128 + f)
        gsums = np.empty((NGRAN, D))
        gsq = np.empty((NGRAN, D))
        gsums[:, :DPE] = st[:, :, :, P].transpose(0, 2, 1).reshape(NGRAN, DPE)
        gsq[:, :DPE] = np.diagonal(
            st[:, :, :, :P], axis1=1, axis2=3).reshape(NGRAN, DPE)
        sq3 = res.results[k]["sq3"].astype(np.float64)   # [128, 8]
        sm3 = res.results[k]["sm3"].astype(np.float64)   # [128, 64]
        gsq[:, DPE:] = (sq3[:, 0::2] + sq3[:, 1::2]).T
        gsums[:, DPE:] = sm3.reshape(P, NGRAN, NCHK).sum(axis=2).T
        for g in range(NGRAN):
            c = gcls[g]
            if c >= 0:
                sums[c] += gsums[g]
                sumsq[c] += gsq[g]
    return sums, sumsq, res


def _pairwise_loss(counts, sums, sumsq, d):
    """The tiny O(C^2 D) stage on host CPU.

    Runs in float32 with the same jax ops as the reference: at these extreme
    betainc parameters (b ~ 8190, x ~ 1e-5) jax's f32 betainc differs from
    the true (f64) value by ~1e-3, so matching the reference requires
    replicating its f32 numerics, not improving on them.
    """
    import jax
    import jax.numpy as jnp

    cpu = jax.devices("cpu")[0]
    with jax.default_device(cpu):
        counts64 = counts.astype(np.float64)
        means64 = sums / counts64[:, None]
        withins64 = sumsq - counts64[:, None] * means64**2
        counts = jnp.asarray(counts64, jnp.float32)               # [C]
        means = jnp.asarray(means64, jnp.float32)                 # [C, D]
        withins = jnp.asarray(withins64, jnp.float32)             # [C, D]
        half_diff = (means[:, None, :] - means[None, :, :]) * 0.5
        pair_counts = counts[:, None] + counts[None, :]
        pair_between = half_diff * half_diff * pair_counts[:, :, None]
        pair_within = withins[:, None, :] + withins[None, :, :]
        d2 = pair_counts - 2.0
        d2 = jnp.where(d2 == 0.0, 1e-5, d2)
        x = pair_between / (pair_between + pair_within)
        x = jnp.clip(x, XMIN, XMAX)
        a = jnp.full_like(x, 0.5)
        b = jnp.broadcast_to((d2 * 0.5)[:, :, None], x.shape)
        xbetainc = jax.scipy.special.betainc(a, b, x)             # [C, C, D]
        top_k, _ = jax.lax.top_k(xbetainc, int(d))                # [C, C, d]
        per_pair = jnp.sum(jnp.log(top_k), axis=-1)               # [C, C]
        mask = jnp.triu(jnp.ones((C, C), dtype=bool), k=1)
        total = jnp.sum(jnp.where(mask, per_pair, jnp.zeros_like(per_pair)))
        return float(-total)


def kernel(hidden, batch_ids, d):
    hidden = np.asarray(hidden, dtype=np.float32)
    ids = np.asarray(batch_ids).astype(np.int64)
    assert hidden.shape == (N, D), hidden.shape

    counts = np.bincount(ids, minlength=C).astype(np.float64)
    sums, sumsq, _ = _device_stats(hidden, ids)
    total = _pairwise_loss(counts, sums, sumsq, int(np.asarray(d)))
    return np.array(total, dtype=np.float32)
